# revision 1
# baseline (speedup 1.0000x reference)
"""2-layer GCN (DGI encoder) as a distributed Bass kernel on 8 TRN2 NeuronCores.

Formulation (per layer, self-loops folded in as ordinary edges):
    out[d] = relu( (sum_{e: dst(e)=d} norm_e * table[src(e)]) @ W + b )
with norm_e = dinv[src]*dinv[dst], dinv = 1/sqrt(in_degree+1).

Distribution: destination nodes are sharded across the 8 cores. Each core's
nodes are packed into PAIRS of 256 output "slots" (load-balanced by degree).
Edges are bucketed by (dest pair, source half) and padded to a uniform tile
count so the SPMD program is identical on every core; per-core data differs
only in the input tensors.

Device pipeline per bucket:
  dma_gather   : pull src rows (512B) from HBM into G[e=128, T, feat=128]
  tensor_scalar: S[e, d] = (iota[d] == destpos_e) * norm_e        (DVE)
  matmul f32r  : psum[feat, 256] += G_t.T @ S_t                   (PE)
per pair:
  matmul fp32  : psum2[dest128, hid] = agg[:, k].T @ W            (PE)
  add bias + relu -> slot-major rows -> DRAM                      (DVE+ACT)
Layer 1 gathers from x itself; an AllGather publishes layer-1 outputs as the
layer-2 gather table.
"""

import math
import os

import numpy as np

from concourse import bacc, bass, mybir
import concourse.tile as tile
from concourse.bass_utils import run_bass_kernel_spmd

F32 = mybir.dt.float32
F32R = mybir.dt.float32r
I16 = mybir.dt.int16

# ---------------- problem config (hardcoded per contract) ----------------
N_NODES = 50000
N_EDGES = 800000
NFEAT = 128
N_CORES = 8
PAIR_SLOTS = 256  # output slots per PSUM window pair
P = 128
GATHER_CHUNK = 8  # tiles (x128 idxs) per dma_gather call (descriptor-ring cap)


class Cfg:
    def __init__(self, n_nodes, n_cores, pair_slots=PAIR_SLOTS, feat=NFEAT):
        self.n_nodes = n_nodes
        self.n_cores = n_cores
        self.feat = feat
        self.shard = n_nodes // n_cores
        assert n_nodes % n_cores == 0
        self.pair_slots = pair_slots
        self.n_pairs = math.ceil(self.shard / pair_slots)
        self.slots_per_core = self.n_pairs * pair_slots
        self.total_slots = self.slots_per_core * n_cores
        # layer-1 gather table is x itself, halved by node id (int16 limit)
        self.half_id = (n_nodes + 1) // 2
        assert max(self.half_id, n_nodes - self.half_id) < 32768
        # layer-2 gather table is the all-gathered slot table, halved by core
        self.half_cores = n_cores // 2
        self.half_rows2 = self.half_cores * self.slots_per_core
        assert self.half_rows2 < 32768 and self.total_slots - self.half_rows2 < 32768
        self.nb = self.n_pairs * 2  # buckets per layer per core


# ---------------- host-side planning ----------------

def _balance_pairs(cfg, wA, wB):
    """Assign each core's nodes to pairs, balancing (halfA, halfB) edge loads.

    Returns slot_of[n_nodes] (slot within owning core) and
    node_of_slot[n_cores, slots_per_core] (-1 for dummy slots)."""
    slot_of = np.full(cfg.n_nodes, -1, np.int64)
    node_of_slot = np.full((cfg.n_cores, cfg.slots_per_core), -1, np.int64)
    for c in range(cfg.n_cores):
        nodes = np.arange(c * cfg.shard, (c + 1) * cfg.shard)
        order = np.argsort(-(wA[nodes] + wB[nodes]), kind="stable")
        loadA = np.zeros(cfg.n_pairs)
        loadB = np.zeros(cfg.n_pairs)
        cnt = np.zeros(cfg.n_pairs, np.int64)
        for n in nodes[order]:
            a, b = wA[n], wB[n]
            score = np.maximum(loadA + a, loadB + b) + (loadA + loadB + a + b) * 1e-4
            score[cnt >= cfg.pair_slots] = np.inf
            p = int(np.argmin(score))
            loadA[p] += a
            loadB[p] += b
            pos = cnt[p]
            cnt[p] += 1
            slot = p * cfg.pair_slots + pos
            slot_of[n] = slot
            node_of_slot[c, slot] = n
    return slot_of, node_of_slot


def _pack_buckets(cfg, bucket_key, n_buckets, idx_vals, pos_vals, norm_vals):
    """Group edges by (core, bucket) and pack padded device arrays.

    bucket_key: per-edge bucket id within its core (0..n_buckets-1)
    idx_vals:   per-edge int gather index (already rebased per half)
    Returns (T, idx_arr[c,128,nb*T*8] i16, dest_arr[c,128,nb*T] f32,
             norm_arr[c,128,nb*T] f32)."""
    ecore = bucket_key[0]
    bkt = bucket_key[1]
    key = ecore * n_buckets + bkt
    order = np.argsort(key, kind="stable")
    key_s = key[order]
    idx_s = idx_vals[order]
    pos_s = pos_vals[order]
    norm_s = norm_vals[order]
    counts = np.bincount(key_s, minlength=cfg.n_cores * n_buckets)
    T = int(math.ceil(counts.max() / P))
    T = max(T, 1)
    L = T * P
    nb = n_buckets
    idx_arr = np.zeros((cfg.n_cores, P, nb * T * 8), np.int16)
    dest_arr = np.full((cfg.n_cores, P, nb * T), -1.0, np.float32)
    norm_arr = np.zeros((cfg.n_cores, P, nb * T), np.float32)
    starts = np.concatenate([[0], np.cumsum(counts)])
    for c in range(cfg.n_cores):
        for b in range(nb):
            k = c * nb + b
            s, e = starts[k], starts[k + 1]
            n = e - s
            I = np.zeros(L, np.int64)
            D = np.full(L, -1.0, np.float32)
            W = np.zeros(L, np.float32)
            I[:n] = idx_s[s:e]
            D[:n] = pos_s[s:e]
            W[:n] = norm_s[s:e]
            # wrapped index layout: wrapped[p, s] = I[s*16 + p], replicated 8x
            wr = I.reshape(T * 8, 16).T.astype(np.int16)  # [16, T*8]
            idx_arr[c, :, b * T * 8:(b + 1) * T * 8] = np.tile(wr, (8, 1))
            dest_arr[c, :, b * T:(b + 1) * T] = D.reshape(T, P).T
            norm_arr[c, :, b * T:(b + 1) * T] = W.reshape(T, P).T
    return T, idx_arr, dest_arr, norm_arr


def plan(cfg, edge_index):
    src = np.asarray(edge_index[0], np.int64)
    dst = np.asarray(edge_index[1], np.int64)
    n = cfg.n_nodes
    deg = np.bincount(dst, minlength=n).astype(np.float64) + 1.0
    dinv = 1.0 / np.sqrt(deg)

    loop = np.arange(n, dtype=np.int64)
    es = np.concatenate([src, loop])  # sources incl self-loops
    ed = np.concatenate([dst, loop])  # dests incl self-loops
    norm = (dinv[es] * dinv[ed]).astype(np.float32)

    # balance pairs on layer-1 half loads (in-edges by source id half + self)
    h1 = (es >= cfg.half_id).astype(np.int64)
    wA = np.bincount(ed[h1 == 0], minlength=n).astype(np.float64)
    wB = np.bincount(ed[h1 == 1], minlength=n).astype(np.float64)
    slot_of, node_of_slot = _balance_pairs(cfg, wA, wB)

    ecore = ed // cfg.shard
    eslot = slot_of[ed]
    epair = eslot // cfg.pair_slots
    epos = (eslot % cfg.pair_slots).astype(np.float32)

    # layer 1: bucket (pair, src-id-half); gather idx = id rebased per half
    b1 = epair * 2 + h1
    idx1v = es - h1 * cfg.half_id
    T1, idx1, dest1, norm1 = _pack_buckets(
        cfg, (ecore, b1), cfg.nb, idx1v, epos, norm)

    # layer 2: bucket (src-core-half, pair); gather idx = table row rebased
    score = es // cfg.shard
    trow = score * cfg.slots_per_core + slot_of[es]
    h2 = (score >= cfg.half_cores).astype(np.int64)
    b2 = h2 * cfg.n_pairs + epair
    idx2v = trow - h2 * cfg.half_rows2
    T2, idx2, dest2, norm2 = _pack_buckets(
        cfg, (ecore, b2), cfg.nb, idx2v, epos, norm)

    return dict(T1=T1, T2=T2, idx1=idx1, dest1=dest1, norm1=norm1,
                idx2=idx2, dest2=dest2, norm2=norm2,
                node_of_slot=node_of_slot)


def make_consts(cfg, W1, W2, b1, b2):
    """[128, 128*4 + pair_slots] fp32: W1 | W2 | b1_bcast | b2_bcast | iota."""
    f = cfg.feat
    consts = np.zeros((P, 4 * f + cfg.pair_slots), np.float32)
    consts[:, 0:f] = W1
    consts[:, f:2 * f] = W2
    consts[:, 2 * f:3 * f] = np.tile(b1[None, :], (P, 1))
    consts[:, 3 * f:4 * f] = np.tile(b2[None, :], (P, 1))
    consts[:, 4 * f:] = np.tile(np.arange(cfg.pair_slots, dtype=np.float32)[None, :],
                                (P, 1))
    return consts


# ---------------- device program ----------------

def build(cfg, T1, T2, dma_scratch=65536):
    # dma_scratch sizes the SWDGE descriptor ring (16B/desc); the default 16KB
    # ring (1024 descs) deadlocks on a single dma_gather with >1024 indices.
    nc = bacc.Bacc(None, target_bir_lowering=False, debug=False,
                   num_devices=cfg.n_cores,
                   dynamic_dma_scratch_size=dma_scratch)
    f = cfg.feat
    ps = cfg.pair_slots
    npair = cfg.n_pairs
    nb = cfg.nb

    x = nc.dram_tensor("x", [cfg.n_nodes, f], F32, kind="ExternalInput")
    idx1 = nc.dram_tensor("idx1", [P, nb * T1 * 8], I16, kind="ExternalInput")
    dest1 = nc.dram_tensor("dest1", [P, nb * T1], F32, kind="ExternalInput")
    norm1 = nc.dram_tensor("norm1", [P, nb * T1], F32, kind="ExternalInput")
    idx2 = nc.dram_tensor("idx2", [P, nb * T2 * 8], I16, kind="ExternalInput")
    dest2 = nc.dram_tensor("dest2", [P, nb * T2], F32, kind="ExternalInput")
    norm2 = nc.dram_tensor("norm2", [P, nb * T2], F32, kind="ExternalInput")
    consts = nc.dram_tensor("consts", [P, 4 * f + ps], F32, kind="ExternalInput")
    z = nc.dram_tensor("z", [cfg.slots_per_core, f], F32, kind="ExternalOutput")

    ag_in = nc.dram_tensor("ag_in", [cfg.slots_per_core, f], F32)
    tab = nc.dram_tensor("tab", [cfg.total_slots, f], F32, addr_space="Shared")

    groups = [list(range(cfg.n_cores))]

    with tile.TileContext(nc) as tc:
        with (
            tc.tile_pool(name="const", bufs=1) as const_pool,
            tc.tile_pool(name="meta", bufs=1) as meta_pool,
            tc.tile_pool(name="gather", bufs=4) as gpool,
            tc.tile_pool(name="onehot", bufs=4) as spool,
            tc.tile_pool(name="agg", bufs=2) as apool,
            tc.tile_pool(name="out", bufs=3) as opool,
            tc.tile_pool(name="accum", bufs=1) as accpool,
            tc.tile_pool(name="psum1", bufs=2, space="PSUM") as psum1_pool,
            tc.tile_pool(name="psum2", bufs=2, space="PSUM") as psum2_pool,
        ):
            ct = const_pool.tile([P, 4 * f + ps], F32)
            nc.sync.dma_start(ct[:], consts[:, :])
            w_t = [ct[:, 0:f], ct[:, f:2 * f]]
            bb_t = [ct[:, 2 * f:3 * f], ct[:, 3 * f:4 * f]]
            iota_t = ct[:, 4 * f:]

            i1t = meta_pool.tile([P, nb * T1 * 8], I16)
            d1t = meta_pool.tile([P, nb * T1], F32)
            n1t = meta_pool.tile([P, nb * T1], F32)
            i2t = meta_pool.tile([P, nb * T2 * 8], I16)
            d2t = meta_pool.tile([P, nb * T2], F32)
            n2t = meta_pool.tile([P, nb * T2], F32)
            nc.sync.dma_start(i1t[:], idx1[:, :])
            nc.sync.dma_start(d1t[:], dest1[:, :])
            nc.sync.dma_start(n1t[:], norm1[:, :])
            nc.sync.dma_start(i2t[:], idx2[:, :])
            nc.sync.dma_start(d2t[:], dest2[:, :])
            nc.sync.dma_start(n2t[:], norm2[:, :])

            accum = accpool.tile([P, npair * ps], F32)

            def do_bucket(psum1, G, it, dt_, nt_, b, T, first, last):
                """All matmuls of one bucket into psum1."""
                for t in range(T):
                    S = spool.tile([P, ps], F32R)
                    col = b * T + t
                    nc.vector.tensor_scalar(
                        S[:], iota_t, dt_[:, col:col + 1], nt_[:, col:col + 1],
                        mybir.AluOpType.is_equal, mybir.AluOpType.mult)
                    nc.tensor.matmul(
                        psum1[:], G[:, t, :], S[:],
                        start=(first and t == 0), stop=(last and t == T - 1),
                        skip_group_check=True)

            def gather_bucket(table_ap, it, b, T):
                """Gather one bucket's src rows, chunked so each dma_gather
                stays under the SWDGE descriptor-ring capacity."""
                G = gpool.tile([P, max(T1, T2), f], F32R, tag="G")
                insts = []
                for t0 in range(0, T, GATHER_CHUNK):
                    tn = min(GATHER_CHUNK, T - t0)
                    insts.append(nc.gpsimd.dma_gather(
                        G[:, t0:t0 + tn, :], table_ap.bitcast(F32R),
                        it[:, (b * T + t0) * 8:(b * T + t0 + tn) * 8],
                        num_idxs=tn * P, num_idxs_reg=tn * P, elem_size=f))
                return G, insts

            def flush_pair(agg, layer, p, out_dram):
                for k in range(ps // P):
                    psum2 = psum2_pool.tile([P, f], F32)
                    nc.tensor.matmul(
                        psum2[:], agg[:, k * P:(k + 1) * P], w_t[layer],
                        start=True, stop=True, skip_group_check=True)
                    ob = opool.tile([P, f], F32, tag="ob")
                    nc.vector.tensor_tensor(
                        ob[:], psum2[:], bb_t[layer], mybir.AluOpType.add)
                    o2 = opool.tile([P, f], F32, tag="o2")
                    nc.scalar.activation(
                        o2[:], ob[:], mybir.ActivationFunctionType.Relu)
                    r0 = p * ps + k * P
                    nc.sync.dma_start(out_dram[r0:r0 + P, :], o2[:])

            # ---------------- layer 1 ----------------
            xh = [x[0:cfg.half_id, :], x[cfg.half_id:cfg.n_nodes, :]]
            for p in range(npair):
                psum1 = psum1_pool.tile([P, ps], F32)
                for h in range(2):
                    b = p * 2 + h
                    G, _ = gather_bucket(xh[h], i1t, b, T1)
                    do_bucket(psum1, G, i1t, d1t, n1t, b, T1,
                              first=(h == 0), last=(h == 1))
                agg = apool.tile([P, ps], F32, tag="agg")
                nc.vector.tensor_copy(agg[:], psum1[:])
                flush_pair(agg, 0, p, ag_in)

            ag_inst = nc.gpsimd.collective_compute(
                "AllGather", mybir.AluOpType.bypass, replica_groups=groups,
                ins=[ag_in[:, :]], outs=[tab[:, :]])

            # ---------------- layer 2 ----------------
            tabh = [tab[0:cfg.half_rows2, :], tab[cfg.half_rows2:cfg.total_slots, :]]
            for h in range(2):
                for p in range(npair):
                    b = h * npair + p
                    psum1 = psum1_pool.tile([P, ps], F32)
                    G, gis = gather_bucket(tabh[h], i2t, b, T2)
                    for gi in gis:
                        bass._add_dep_helper(gi.ins, ag_inst.ins,
                                             reason="gather after AG")
                    do_bucket(psum1, G, i2t, d2t, n2t, b, T2,
                              first=True, last=True)
                    if h == 0:
                        nc.vector.tensor_copy(
                            accum[:, p * ps:(p + 1) * ps], psum1[:])
                    else:
                        agg = apool.tile([P, ps], F32, tag="agg")
                        nc.vector.tensor_tensor(
                            agg[:], psum1[:], accum[:, p * ps:(p + 1) * ps],
                            mybir.AluOpType.add)
                        flush_pair(agg, 1, p, z)

    nc.compile()
    return nc


# ---------------- top level ----------------

def run(cfg, x, edge_index, W1, b1, W2, b2, trace=False):
    pl = plan(cfg, edge_index)
    nc = build(cfg, pl["T1"], pl["T2"])
    consts = make_consts(cfg, np.asarray(W1, np.float32), np.asarray(W2, np.float32),
                         np.asarray(b1, np.float32), np.asarray(b2, np.float32))
    x = np.ascontiguousarray(np.asarray(x, np.float32))
    in_maps = []
    for c in range(cfg.n_cores):
        in_maps.append({
            "x": x,
            "idx1": np.ascontiguousarray(pl["idx1"][c]),
            "dest1": np.ascontiguousarray(pl["dest1"][c]),
            "norm1": np.ascontiguousarray(pl["norm1"][c]),
            "idx2": np.ascontiguousarray(pl["idx2"][c]),
            "dest2": np.ascontiguousarray(pl["dest2"][c]),
            "norm2": np.ascontiguousarray(pl["norm2"][c]),
            "consts": consts,
        })
    res = run_bass_kernel_spmd(nc, in_maps, list(range(cfg.n_cores)), trace=trace)
    out = np.empty((cfg.n_nodes, cfg.feat), np.float32)
    for c in range(cfg.n_cores):
        zc = res.results[c]["z"]
        sel = pl["node_of_slot"][c]
        valid = sel >= 0
        out[sel[valid]] = zc[valid]
    return out, res


def kernel(x, edge_index, W1, b1, W2, b2):
    cfg = Cfg(N_NODES, N_CORES)
    out, _ = run(cfg, x, edge_index, W1, b1, W2, b2, trace=False)
    return out



# revision 9
# speedup vs baseline: 1.7930x; 1.7930x over previous
"""2-layer GCN (DGI encoder) as a distributed Bass kernel on 8 TRN2 NeuronCores.

Formulation (per layer, self-loops folded in as ordinary edges):
    out[d] = relu( (sum_{e: dst(e)=d} norm_e * table[src(e)]) @ W + b )
with norm_e = dinv[src]*dinv[dst], dinv = 1/sqrt(in_degree+1).

Distribution: destination nodes are sharded across the 8 cores. Each core's
nodes are packed into PAIRS of 256 output "slots" (load-balanced by degree).
Edges are bucketed by (dest pair, source half) and padded to a uniform tile
count so the SPMD program is identical on every core; per-core data differs
only in the input tensors.

Device pipeline per bucket:
  dma_gather   : pull src rows (512B) from HBM into G[e=128, T, feat=128]
  tensor_scalar: S[e, d] = (iota[d] == destpos_e) * norm_e        (DVE)
  matmul f32r  : psum[feat, 256] += G_t.T @ S_t                   (PE)
per pair:
  matmul fp32  : psum2[dest128, hid] = agg[:, k].T @ W            (PE)
  add bias + relu -> slot-major rows -> DRAM                      (DVE+ACT)
Layer 1 gathers from x itself; an AllGather publishes layer-1 outputs as the
layer-2 gather table.
"""

import math
import os

import numpy as np

from concourse import bacc, bass, mybir
import concourse.tile as tile
from concourse.bass_utils import run_bass_kernel_spmd

F32 = mybir.dt.float32
F32R = mybir.dt.float32r
BF16 = mybir.dt.bfloat16
I16 = mybir.dt.int16

# ---------------- problem config (hardcoded per contract) ----------------
N_NODES = 50000
N_EDGES = 800000
NFEAT = 128
N_CORES = 8
PAIR_SLOTS = 256  # output slots per PSUM window pair
P = 128
GATHER_CHUNK = 8  # tiles (x128 idxs) per dma_gather call (descriptor-ring cap)
N_QUEUES = 4  # SWDGE queues; each runs desc-gen on its own Q7 core pair


class Cfg:
    def __init__(self, n_nodes, n_cores, pair_slots=PAIR_SLOTS, feat=NFEAT):
        self.n_nodes = n_nodes
        self.n_cores = n_cores
        self.feat = feat
        self.shard = n_nodes // n_cores
        assert n_nodes % n_cores == 0
        self.pair_slots = pair_slots
        self.n_pairs = math.ceil(self.shard / pair_slots)
        self.slots_per_core = self.n_pairs * pair_slots
        self.total_slots = self.slots_per_core * n_cores
        # layer-1 gather table is x itself, halved by node id (int16 limit)
        self.half_id = (n_nodes + 1) // 2
        assert max(self.half_id, n_nodes - self.half_id) < 32768
        # layer-2 gather table is the all-gathered slot table, halved by core
        self.half_cores = n_cores // 2
        self.half_rows2 = self.half_cores * self.slots_per_core
        assert self.half_rows2 < 32768 and self.total_slots - self.half_rows2 < 32768
        self.nb = self.n_pairs * 2  # buckets per layer per core


# ---------------- host-side planning ----------------

def _balance_pairs(cfg, wA, wB):
    """Assign each core's nodes to pairs, balancing (halfA, halfB) edge loads.

    Returns slot_of[n_nodes] (slot within owning core) and
    node_of_slot[n_cores, slots_per_core] (-1 for dummy slots)."""
    slot_of = np.full(cfg.n_nodes, -1, np.int64)
    node_of_slot = np.full((cfg.n_cores, cfg.slots_per_core), -1, np.int64)
    for c in range(cfg.n_cores):
        nodes = np.arange(c * cfg.shard, (c + 1) * cfg.shard)
        order = np.argsort(-(wA[nodes] + wB[nodes]), kind="stable")
        loadA = np.zeros(cfg.n_pairs)
        loadB = np.zeros(cfg.n_pairs)
        cnt = np.zeros(cfg.n_pairs, np.int64)
        for n in nodes[order]:
            a, b = wA[n], wB[n]
            score = np.maximum(loadA + a, loadB + b) + (loadA + loadB + a + b) * 1e-4
            score[cnt >= cfg.pair_slots] = np.inf
            p = int(np.argmin(score))
            loadA[p] += a
            loadB[p] += b
            pos = cnt[p]
            cnt[p] += 1
            slot = p * cfg.pair_slots + pos
            slot_of[n] = slot
            node_of_slot[c, slot] = n
    return slot_of, node_of_slot


def _pack_buckets(cfg, bucket_key, n_buckets, idx_vals, pos_vals, norm_vals):
    """Group edges by (core, bucket) and pack padded device arrays.

    bucket_key: per-edge bucket id within its core (0..n_buckets-1)
    idx_vals:   per-edge int gather index (already rebased per half)
    Returns (T, idx_arr[c,128,nb*T*8] i16, dest_arr[c,128,nb*T] f32,
             norm_arr[c,128,nb*T] f32)."""
    ecore = bucket_key[0]
    bkt = bucket_key[1]
    key = ecore * n_buckets + bkt
    order = np.argsort(key, kind="stable")
    key_s = key[order]
    idx_s = idx_vals[order]
    pos_s = pos_vals[order]
    norm_s = norm_vals[order]
    counts = np.bincount(key_s, minlength=cfg.n_cores * n_buckets)
    T = int(math.ceil(counts.max() / P))
    T = max(T, 1)
    L = T * P
    nb = n_buckets
    idx_arr = np.zeros((cfg.n_cores, P, nb * T * 8), np.int16)
    dest_arr = np.full((cfg.n_cores, P, nb * T), -1.0, np.float32)
    norm_arr = np.zeros((cfg.n_cores, P, nb * T), np.float32)
    starts = np.concatenate([[0], np.cumsum(counts)])
    for c in range(cfg.n_cores):
        for b in range(nb):
            k = c * nb + b
            s, e = starts[k], starts[k + 1]
            n = e - s
            I = np.zeros(L, np.int64)
            D = np.full(L, -1.0, np.float32)
            W = np.zeros(L, np.float32)
            I[:n] = idx_s[s:e]
            D[:n] = pos_s[s:e]
            W[:n] = norm_s[s:e]
            # wrapped index layout: wrapped[p, s] = I[s*16 + p], replicated 8x
            wr = I.reshape(T * 8, 16).T.astype(np.int16)  # [16, T*8]
            idx_arr[c, :, b * T * 8:(b + 1) * T * 8] = np.tile(wr, (8, 1))
            dest_arr[c, :, b * T:(b + 1) * T] = D.reshape(T, P).T
            norm_arr[c, :, b * T:(b + 1) * T] = W.reshape(T, P).T
    return T, idx_arr, dest_arr, norm_arr


def plan(cfg, edge_index):
    src = np.asarray(edge_index[0], np.int64)
    dst = np.asarray(edge_index[1], np.int64)
    n = cfg.n_nodes
    deg = np.bincount(dst, minlength=n).astype(np.float64) + 1.0
    dinv = 1.0 / np.sqrt(deg)

    loop = np.arange(n, dtype=np.int64)
    es = np.concatenate([src, loop])  # sources incl self-loops
    ed = np.concatenate([dst, loop])  # dests incl self-loops
    norm = (dinv[es] * dinv[ed]).astype(np.float32)

    # balance pairs on layer-1 half loads (in-edges by source id half + self)
    h1 = (es >= cfg.half_id).astype(np.int64)
    wA = np.bincount(ed[h1 == 0], minlength=n).astype(np.float64)
    wB = np.bincount(ed[h1 == 1], minlength=n).astype(np.float64)
    slot_of, node_of_slot = _balance_pairs(cfg, wA, wB)

    ecore = ed // cfg.shard
    eslot = slot_of[ed]
    epair = eslot // cfg.pair_slots
    epos = (eslot % cfg.pair_slots).astype(np.float32)

    # layer 1: bucket (pair, src-id-half); gather idx = id rebased per half
    b1 = epair * 2 + h1
    idx1v = es - h1 * cfg.half_id
    T1, idx1, dest1, norm1 = _pack_buckets(
        cfg, (ecore, b1), cfg.nb, idx1v, epos, norm)

    # layer 2: bucket (src-core-half, pair); gather idx = table row rebased
    score = es // cfg.shard
    trow = score * cfg.slots_per_core + slot_of[es]
    h2 = (score >= cfg.half_cores).astype(np.int64)
    b2 = h2 * cfg.n_pairs + epair
    idx2v = trow - h2 * cfg.half_rows2
    T2, idx2, dest2, norm2 = _pack_buckets(
        cfg, (ecore, b2), cfg.nb, idx2v, epos, norm)

    return dict(T1=T1, T2=T2, idx1=idx1, dest1=dest1, norm1=norm1,
                idx2=idx2, dest2=dest2, norm2=norm2,
                node_of_slot=node_of_slot)


def make_consts(cfg, W1, W2, b1, b2):
    """cbf [128, 2f + pair_slots] bf16: W1 | W2 | iota.
    cf32 [128, 2f] fp32: b1_bcast | b2_bcast."""
    import ml_dtypes
    f = cfg.feat
    cbf = np.zeros((P, 2 * f + cfg.pair_slots), ml_dtypes.bfloat16)
    cbf[:, 0:f] = W1.astype(ml_dtypes.bfloat16)
    cbf[:, f:2 * f] = W2.astype(ml_dtypes.bfloat16)
    cbf[:, 2 * f:] = np.tile(
        np.arange(cfg.pair_slots, dtype=np.float32)[None, :], (P, 1)
    ).astype(ml_dtypes.bfloat16)
    cf32 = np.zeros((P, 2 * f), np.float32)
    cf32[:, 0:f] = np.tile(b1[None, :], (P, 1))
    cf32[:, f:2 * f] = np.tile(b2[None, :], (P, 1))
    return cbf, cf32


# ---------------- device program ----------------

def build(cfg, T1, T2, dma_scratch=65536):
    # dma_scratch sizes the SWDGE descriptor ring (16B/desc); the default 16KB
    # ring (1024 descs) deadlocks on a single dma_gather with >1024 indices.
    nc = bacc.Bacc(None, target_bir_lowering=False, debug=False,
                   num_devices=cfg.n_cores,
                   dynamic_dma_scratch_size=dma_scratch,
                   num_swdge_queues=N_QUEUES)
    f = cfg.feat
    ps = cfg.pair_slots
    npair = cfg.n_pairs
    nb = cfg.nb

    x = nc.dram_tensor("x", [cfg.n_nodes, f], F32, kind="ExternalInput")
    idx1 = nc.dram_tensor("idx1", [P, nb * T1 * 8], I16, kind="ExternalInput")
    dest1 = nc.dram_tensor("dest1", [P, nb * T1], F32, kind="ExternalInput")
    norm1 = nc.dram_tensor("norm1", [P, nb * T1], F32, kind="ExternalInput")
    idx2 = nc.dram_tensor("idx2", [P, nb * T2 * 8], I16, kind="ExternalInput")
    dest2 = nc.dram_tensor("dest2", [P, nb * T2], F32, kind="ExternalInput")
    norm2 = nc.dram_tensor("norm2", [P, nb * T2], F32, kind="ExternalInput")
    cbf = nc.dram_tensor("cbf", [P, 2 * f + ps], BF16, kind="ExternalInput")
    cf32 = nc.dram_tensor("cf32", [P, 2 * f], F32, kind="ExternalInput")
    z = nc.dram_tensor("z", [cfg.slots_per_core, f], F32, kind="ExternalOutput")

    xbf = nc.dram_tensor("xbf", [cfg.n_nodes, f], BF16)
    ag_in = nc.dram_tensor("ag_in", [cfg.slots_per_core, f], BF16)
    tab = nc.dram_tensor("tab", [cfg.total_slots, f], BF16, addr_space="Shared")

    groups = [list(range(cfg.n_cores))]

    with tile.TileContext(nc) as tc:
        with (
            tc.tile_pool(name="const", bufs=1) as const_pool,
            tc.tile_pool(name="meta", bufs=1) as meta_pool,
            tc.tile_pool(name="gather", bufs=4) as gpool,
            tc.tile_pool(name="onehot", bufs=4) as spool,
            tc.tile_pool(name="agg", bufs=2) as apool,
            tc.tile_pool(name="out", bufs=3) as opool,
            tc.tile_pool(name="accum", bufs=1) as accpool,
            tc.tile_pool(name="psum1", bufs=2, space="PSUM") as psum1_pool,
            tc.tile_pool(name="psum2", bufs=2, space="PSUM") as psum2_pool,
        ):
            ct = const_pool.tile([P, 2 * f + ps], BF16)
            nc.sync.dma_start(ct[:], cbf[:, :])
            ct32 = const_pool.tile([P, 2 * f], F32)
            nc.sync.dma_start(ct32[:], cf32[:, :])
            w_t = [ct[:, 0:f], ct[:, f:2 * f]]
            bb_t = [ct32[:, 0:f], ct32[:, f:2 * f]]
            iota_t = ct[:, 2 * f:]

            # cast x (fp32) -> xbf (bf16) in DRAM, halved so layer-1 gathers
            # of half h can start as soon as cast h lands
            cast_insts = []
            for h in range(2):
                r0 = 0 if h == 0 else cfg.half_id
                r1 = cfg.half_id if h == 0 else cfg.n_nodes
                cast_insts.append(
                    nc.gpsimd.dma_start(xbf[r0:r1, :], x[r0:r1, :]))

            i1t = meta_pool.tile([P, nb * T1 * 8], I16)
            d1t = meta_pool.tile([P, nb * T1], F32)
            n1t = meta_pool.tile([P, nb * T1], F32)
            i2t = meta_pool.tile([P, nb * T2 * 8], I16)
            d2t = meta_pool.tile([P, nb * T2], F32)
            n2t = meta_pool.tile([P, nb * T2], F32)
            nc.sync.dma_start(i1t[:], idx1[:, :])
            nc.sync.dma_start(d1t[:], dest1[:, :])
            nc.sync.dma_start(n1t[:], norm1[:, :])
            nc.sync.dma_start(i2t[:], idx2[:, :])
            nc.sync.dma_start(d2t[:], dest2[:, :])
            nc.sync.dma_start(n2t[:], norm2[:, :])

            accum = accpool.tile([P, npair * ps], F32)

            def do_bucket(psum1, G, it, dt_, nt_, b, T, first, last):
                """All matmuls of one bucket into psum1."""
                for t in range(T):
                    S = spool.tile([P, ps], F32R)
                    col = b * T + t
                    nc.vector.tensor_scalar(
                        S[:], iota_t, dt_[:, col:col + 1], nt_[:, col:col + 1],
                        mybir.AluOpType.is_equal, mybir.AluOpType.mult)
                    nc.tensor.matmul(
                        psum1[:], G[:, t, :], S[:],
                        start=(first and t == 0), stop=(last and t == T - 1),
                        skip_group_check=True)

            qctr = [0]

            def gather_bucket(table_ap, it, b, T):
                """Gather one bucket's src rows, chunked so each dma_gather
                stays under the SWDGE descriptor-ring capacity. Calls rotate
                across SWDGE queues so desc-gen runs on all Q7 core pairs."""
                G = gpool.tile([P, max(T1, T2), f], F32R, tag="G")
                insts = []
                for t0 in range(0, T, GATHER_CHUNK):
                    tn = min(GATHER_CHUNK, T - t0)
                    q = qctr[0] % N_QUEUES
                    qctr[0] += 1
                    insts.append(nc.gpsimd.dma_gather(
                        G[:, t0:t0 + tn, :], table_ap.bitcast(F32R),
                        it[:, (b * T + t0) * 8:(b * T + t0 + tn) * 8],
                        num_idxs=tn * P, num_idxs_reg=tn * P, elem_size=f,
                        queue_num=q))
                return G, insts

            def flush_pair(agg, layer, p, out_dram):
                for k in range(ps // P):
                    psum2 = psum2_pool.tile([P, f], F32)
                    nc.tensor.matmul(
                        psum2[:], agg[:, k * P:(k + 1) * P], w_t[layer],
                        start=True, stop=True, skip_group_check=True)
                    ob = opool.tile([P, f], F32, tag="ob")
                    nc.vector.tensor_tensor(
                        ob[:], psum2[:], bb_t[layer], mybir.AluOpType.add)
                    o2 = opool.tile([P, f], F32, tag="o2")
                    nc.scalar.activation(
                        o2[:], ob[:], mybir.ActivationFunctionType.Relu)
                    r0 = p * ps + k * P
                    nc.sync.dma_start(out_dram[r0:r0 + P, :], o2[:])

            # ---------------- layer 1 ----------------
            xh = [x[0:cfg.half_id, :], x[cfg.half_id:cfg.n_nodes, :]]
            for p in range(npair):
                psum1 = psum1_pool.tile([P, ps], F32)
                for h in range(2):
                    b = p * 2 + h
                    G, _ = gather_bucket(xh[h], i1t, b, T1)
                    do_bucket(psum1, G, i1t, d1t, n1t, b, T1,
                              first=(h == 0), last=(h == 1))
                agg = apool.tile([P, ps], F32, tag="agg")
                nc.vector.tensor_copy(agg[:], psum1[:])
                flush_pair(agg, 0, p, ag_in)

            ag_inst = nc.gpsimd.collective_compute(
                "AllGather", mybir.AluOpType.bypass, replica_groups=groups,
                ins=[ag_in[:, :]], outs=[tab[:, :]])

            # ---------------- layer 2 ----------------
            tabh = [tab[0:cfg.half_rows2, :], tab[cfg.half_rows2:cfg.total_slots, :]]
            for h in range(2):
                for p in range(npair):
                    b = h * npair + p
                    psum1 = psum1_pool.tile([P, ps], F32)
                    G, gis = gather_bucket(tabh[h], i2t, b, T2)
                    for gi in gis:
                        bass._add_dep_helper(gi.ins, ag_inst.ins,
                                             reason="gather after AG")
                    do_bucket(psum1, G, i2t, d2t, n2t, b, T2,
                              first=True, last=True)
                    if h == 0:
                        nc.vector.tensor_copy(
                            accum[:, p * ps:(p + 1) * ps], psum1[:])
                    else:
                        agg = apool.tile([P, ps], F32, tag="agg")
                        nc.vector.tensor_tensor(
                            agg[:], psum1[:], accum[:, p * ps:(p + 1) * ps],
                            mybir.AluOpType.add)
                        flush_pair(agg, 1, p, z)

    nc.compile()
    return nc


# ---------------- top level ----------------

def run(cfg, x, edge_index, W1, b1, W2, b2, trace=False):
    pl = plan(cfg, edge_index)
    nc = build(cfg, pl["T1"], pl["T2"])
    consts = make_consts(cfg, np.asarray(W1, np.float32), np.asarray(W2, np.float32),
                         np.asarray(b1, np.float32), np.asarray(b2, np.float32))
    x = np.ascontiguousarray(np.asarray(x, np.float32))
    in_maps = []
    for c in range(cfg.n_cores):
        in_maps.append({
            "x": x,
            "idx1": np.ascontiguousarray(pl["idx1"][c]),
            "dest1": np.ascontiguousarray(pl["dest1"][c]),
            "norm1": np.ascontiguousarray(pl["norm1"][c]),
            "idx2": np.ascontiguousarray(pl["idx2"][c]),
            "dest2": np.ascontiguousarray(pl["dest2"][c]),
            "norm2": np.ascontiguousarray(pl["norm2"][c]),
            "consts": consts,
        })
    res = run_bass_kernel_spmd(nc, in_maps, list(range(cfg.n_cores)), trace=trace)
    out = np.empty((cfg.n_nodes, cfg.feat), np.float32)
    for c in range(cfg.n_cores):
        zc = res.results[c]["z"]
        sel = pl["node_of_slot"][c]
        valid = sel >= 0
        out[sel[valid]] = zc[valid]
    return out, res


def kernel(x, edge_index, W1, b1, W2, b2):
    cfg = Cfg(N_NODES, N_CORES)
    out, _ = run(cfg, x, edge_index, W1, b1, W2, b2, trace=False)
    return out



# revision 15
# speedup vs baseline: 1.9162x; 1.0687x over previous
"""2-layer GCN (DGI encoder) as a distributed Bass kernel on 8 TRN2 NeuronCores.

Formulation (per layer, self-loops folded in as ordinary edges):
    out[d] = relu( (sum_{e: dst(e)=d} norm_e * table[src(e)]) @ W + b )
with norm_e = dinv[src]*dinv[dst], dinv = 1/sqrt(in_degree+1).

Distribution: destination nodes are sharded across the 8 cores. Each core's
nodes are packed into PAIRS of 256 output "slots" (load-balanced by degree).
Edges are bucketed by (dest pair, source half) and padded to a uniform tile
count so the SPMD program is identical on every core; per-core data differs
only in the input tensors.

Device pipeline per bucket:
  dma_gather   : pull src rows (512B) from HBM into G[e=128, T, feat=128]
  tensor_scalar: S[e, d] = (iota[d] == destpos_e) * norm_e        (DVE)
  matmul f32r  : psum[feat, 256] += G_t.T @ S_t                   (PE)
per pair:
  matmul fp32  : psum2[dest128, hid] = agg[:, k].T @ W            (PE)
  add bias + relu -> slot-major rows -> DRAM                      (DVE+ACT)
Layer 1 gathers from x itself; an AllGather publishes layer-1 outputs as the
layer-2 gather table.
"""

import math
import os

import numpy as np

from concourse import bacc, bass, mybir
import concourse.tile as tile
from concourse.bass_utils import run_bass_kernel_spmd

F32 = mybir.dt.float32
F32R = mybir.dt.float32r
BF16 = mybir.dt.bfloat16
I16 = mybir.dt.int16

# ---------------- problem config (hardcoded per contract) ----------------
N_NODES = 50000
N_EDGES = 800000
NFEAT = 128
N_CORES = 8
PAIR_SLOTS = 256  # output slots per PSUM window pair
P = 128
GATHER_CHUNK = 8  # tiles (x128 idxs) per dma_gather call (descriptor-ring cap)
N_QUEUES = 4  # SWDGE queues; each runs desc-gen on its own Q7 core pair


class Cfg:
    def __init__(self, n_nodes, n_cores, pair_slots=PAIR_SLOTS, feat=NFEAT):
        self.n_nodes = n_nodes
        self.n_cores = n_cores
        self.feat = feat
        self.shard = n_nodes // n_cores
        assert n_nodes % n_cores == 0
        self.pair_slots = pair_slots
        self.n_pairs = math.ceil(self.shard / pair_slots)
        self.slots_per_core = self.n_pairs * pair_slots
        self.total_slots = self.slots_per_core * n_cores
        # layer-1 gather table is x itself, halved by node id (int16 limit)
        self.half_id = (n_nodes + 1) // 2
        assert max(self.half_id, n_nodes - self.half_id) < 32768
        # layer-2 gather table is the all-gathered slot table, halved by core
        self.half_cores = n_cores // 2
        self.half_rows2 = self.half_cores * self.slots_per_core
        assert self.half_rows2 < 32768 and self.total_slots - self.half_rows2 < 32768
        self.nb = self.n_pairs * 2  # buckets per layer per core


# ---------------- host-side planning ----------------

def _balance_pairs(cfg, wA, wB):
    """Assign each core's nodes to pairs, balancing (halfA, halfB) edge loads.

    Returns slot_of[n_nodes] (slot within owning core) and
    node_of_slot[n_cores, slots_per_core] (-1 for dummy slots)."""
    slot_of = np.full(cfg.n_nodes, -1, np.int64)
    node_of_slot = np.full((cfg.n_cores, cfg.slots_per_core), -1, np.int64)
    for c in range(cfg.n_cores):
        nodes = np.arange(c * cfg.shard, (c + 1) * cfg.shard)
        order = np.argsort(-(wA[nodes] + wB[nodes]), kind="stable")
        loadA = np.zeros(cfg.n_pairs)
        loadB = np.zeros(cfg.n_pairs)
        cnt = np.zeros(cfg.n_pairs, np.int64)
        for n in nodes[order]:
            a, b = wA[n], wB[n]
            score = np.maximum(loadA + a, loadB + b) + (loadA + loadB + a + b) * 1e-4
            score[cnt >= cfg.pair_slots] = np.inf
            p = int(np.argmin(score))
            loadA[p] += a
            loadB[p] += b
            pos = cnt[p]
            cnt[p] += 1
            slot = p * cfg.pair_slots + pos
            slot_of[n] = slot
            node_of_slot[c, slot] = n
    return slot_of, node_of_slot


def _pack_buckets(cfg, bucket_key, n_buckets, idx_vals, pos_vals, norm_vals):
    """Group edges by (core, bucket) and pack padded device arrays.

    bucket_key: per-edge bucket id within its core (0..n_buckets-1)
    idx_vals:   per-edge int gather index (already rebased per half)
    Returns (T, idx_arr[c,128,nb*T*8] i16, dest_arr[c,128,nb*T] f32,
             norm_arr[c,128,nb*T] f32)."""
    ecore = bucket_key[0]
    bkt = bucket_key[1]
    key = ecore * n_buckets + bkt
    order = np.argsort(key, kind="stable")
    key_s = key[order]
    idx_s = idx_vals[order]
    pos_s = pos_vals[order]
    norm_s = norm_vals[order]
    counts = np.bincount(key_s, minlength=cfg.n_cores * n_buckets)
    T = int(math.ceil(counts.max() / P))
    T = max(T, 1)
    L = T * P
    nb = n_buckets
    idx_arr = np.zeros((cfg.n_cores, P, nb * T * 8), np.int16)
    dest_arr = np.full((cfg.n_cores, P, nb * T), -1.0, np.float32)
    norm_arr = np.zeros((cfg.n_cores, P, nb * T), np.float32)
    starts = np.concatenate([[0], np.cumsum(counts)])
    for c in range(cfg.n_cores):
        for b in range(nb):
            k = c * nb + b
            s, e = starts[k], starts[k + 1]
            n = e - s
            I = np.zeros(L, np.int64)
            D = np.full(L, -1.0, np.float32)
            W = np.zeros(L, np.float32)
            I[:n] = idx_s[s:e]
            D[:n] = pos_s[s:e]
            W[:n] = norm_s[s:e]
            # wrapped index layout: wrapped[p, s] = I[s*16 + p], replicated 8x
            wr = I.reshape(T * 8, 16).T.astype(np.int16)  # [16, T*8]
            idx_arr[c, :, b * T * 8:(b + 1) * T * 8] = np.tile(wr, (8, 1))
            dest_arr[c, :, b * T:(b + 1) * T] = D.reshape(T, P).T
            norm_arr[c, :, b * T:(b + 1) * T] = W.reshape(T, P).T
    return T, idx_arr, dest_arr, norm_arr


def plan(cfg, edge_index):
    src = np.asarray(edge_index[0], np.int64)
    dst = np.asarray(edge_index[1], np.int64)
    n = cfg.n_nodes
    deg = np.bincount(dst, minlength=n).astype(np.float64) + 1.0
    dinv = 1.0 / np.sqrt(deg)

    loop = np.arange(n, dtype=np.int64)
    es = np.concatenate([src, loop])  # sources incl self-loops
    ed = np.concatenate([dst, loop])  # dests incl self-loops
    norm = (dinv[es] * dinv[ed]).astype(np.float32)

    # balance pairs on layer-1 half loads (in-edges by source id half + self)
    h1 = (es >= cfg.half_id).astype(np.int64)
    wA = np.bincount(ed[h1 == 0], minlength=n).astype(np.float64)
    wB = np.bincount(ed[h1 == 1], minlength=n).astype(np.float64)
    slot_of, node_of_slot = _balance_pairs(cfg, wA, wB)

    ecore = ed // cfg.shard
    eslot = slot_of[ed]
    epair = eslot // cfg.pair_slots
    epos = (eslot % cfg.pair_slots).astype(np.float32)

    # layer 1: bucket (pair, src-id-half); gather idx = id rebased per half
    b1 = epair * 2 + h1
    idx1v = es - h1 * cfg.half_id
    T1, idx1, dest1, norm1 = _pack_buckets(
        cfg, (ecore, b1), cfg.nb, idx1v, epos, norm)

    # layer 2: bucket (src-core-half, pair); gather idx = table row rebased
    score = es // cfg.shard
    trow = score * cfg.slots_per_core + slot_of[es]
    h2 = (score >= cfg.half_cores).astype(np.int64)
    b2 = h2 * cfg.n_pairs + epair
    idx2v = trow - h2 * cfg.half_rows2
    T2, idx2, dest2, norm2 = _pack_buckets(
        cfg, (ecore, b2), cfg.nb, idx2v, epos, norm)

    return dict(T1=T1, T2=T2, idx1=idx1, dest1=dest1, norm1=norm1,
                idx2=idx2, dest2=dest2, norm2=norm2,
                node_of_slot=node_of_slot)


def make_consts(cfg, W1, W2, b1, b2):
    """cbf [128, 2f + pair_slots] bf16: W1 | W2 | iota.
    cf32 [128, 2f] fp32: b1_bcast | b2_bcast."""
    import ml_dtypes
    f = cfg.feat
    cbf = np.zeros((P, 2 * f + cfg.pair_slots), ml_dtypes.bfloat16)
    cbf[:, 0:f] = W1.astype(ml_dtypes.bfloat16)
    cbf[:, f:2 * f] = W2.astype(ml_dtypes.bfloat16)
    cbf[:, 2 * f:] = np.tile(
        np.arange(cfg.pair_slots, dtype=np.float32)[None, :], (P, 1)
    ).astype(ml_dtypes.bfloat16)
    cf32 = np.zeros((P, 2 * f), np.float32)
    cf32[:, 0:f] = np.tile(b1[None, :], (P, 1))
    cf32[:, f:2 * f] = np.tile(b2[None, :], (P, 1))
    return cbf, cf32


# ---------------- device program ----------------

def build(cfg, T1, T2, dma_scratch=65536):
    # dma_scratch sizes the SWDGE descriptor ring (16B/desc); the default 16KB
    # ring (1024 descs) deadlocks on a single dma_gather with >1024 indices.
    nc = bacc.Bacc(None, target_bir_lowering=False, debug=False,
                   num_devices=cfg.n_cores,
                   dynamic_dma_scratch_size=dma_scratch,
                   num_swdge_queues=N_QUEUES)
    f = cfg.feat
    ps = cfg.pair_slots
    npair = cfg.n_pairs
    nb = cfg.nb

    x = nc.dram_tensor("x", [cfg.n_nodes, f], F32, kind="ExternalInput")
    idx1 = nc.dram_tensor("idx1", [P, nb * T1 * 8], I16, kind="ExternalInput")
    dest1 = nc.dram_tensor("dest1", [P, nb * T1], F32, kind="ExternalInput")
    norm1 = nc.dram_tensor("norm1", [P, nb * T1], F32, kind="ExternalInput")
    idx2 = nc.dram_tensor("idx2", [P, nb * T2 * 8], I16, kind="ExternalInput")
    dest2 = nc.dram_tensor("dest2", [P, nb * T2], F32, kind="ExternalInput")
    norm2 = nc.dram_tensor("norm2", [P, nb * T2], F32, kind="ExternalInput")
    cbf = nc.dram_tensor("cbf", [P, 2 * f + ps], BF16, kind="ExternalInput")
    cf32 = nc.dram_tensor("cf32", [P, 2 * f], F32, kind="ExternalInput")
    z = nc.dram_tensor("z", [cfg.slots_per_core, f], F32, kind="ExternalOutput")

    xbf = nc.dram_tensor("xbf", [cfg.n_nodes, f], BF16)
    ag_in = nc.dram_tensor("ag_in", [cfg.slots_per_core, f], BF16)
    tab = nc.dram_tensor("tab", [cfg.total_slots, f], BF16, addr_space="Shared")

    groups = [list(range(cfg.n_cores))]

    with tile.TileContext(nc) as tc:
        with (
            tc.tile_pool(name="const", bufs=1) as const_pool,
            tc.tile_pool(name="meta", bufs=1) as meta_pool,
            tc.tile_pool(name="gather", bufs=4) as gpool,
            tc.tile_pool(name="onehot", bufs=4) as spool,
            tc.tile_pool(name="agg", bufs=2) as apool,
            tc.tile_pool(name="out", bufs=3) as opool,
            tc.tile_pool(name="accum", bufs=1) as accpool,
            tc.tile_pool(name="psum1", bufs=2, space="PSUM") as psum1_pool,
            tc.tile_pool(name="psum2", bufs=2, space="PSUM") as psum2_pool,
        ):
            ct = const_pool.tile([P, 2 * f + ps], BF16)
            nc.sync.dma_start(ct[:], cbf[:, :])
            ct32 = const_pool.tile([P, 2 * f], F32)
            nc.sync.dma_start(ct32[:], cf32[:, :])
            w_t = [ct[:, 0:f], ct[:, f:2 * f]]
            bb_t = [ct32[:, 0:f], ct32[:, f:2 * f]]
            iota_t = ct[:, 2 * f:]

            # cast x (fp32) -> xbf (bf16) in DRAM, halved so layer-1 gathers
            # of half h can start as soon as cast h lands
            cast_insts = []
            for h in range(2):
                r0 = 0 if h == 0 else cfg.half_id
                r1 = cfg.half_id if h == 0 else cfg.n_nodes
                cast_insts.append(
                    nc.gpsimd.dma_start(xbf[r0:r1, :], x[r0:r1, :]))

            i1t = meta_pool.tile([P, nb * T1 * 8], I16)
            d1t = meta_pool.tile([P, nb * T1], F32)
            n1t = meta_pool.tile([P, nb * T1], F32)
            i2t = meta_pool.tile([P, nb * T2 * 8], I16)
            d2t = meta_pool.tile([P, nb * T2], F32)
            n2t = meta_pool.tile([P, nb * T2], F32)
            nc.sync.dma_start(i1t[:], idx1[:, :])
            nc.sync.dma_start(d1t[:], dest1[:, :])
            nc.sync.dma_start(n1t[:], norm1[:, :])
            nc.sync.dma_start(i2t[:], idx2[:, :])
            nc.sync.dma_start(d2t[:], dest2[:, :])
            nc.sync.dma_start(n2t[:], norm2[:, :])

            accum = accpool.tile([P, npair * ps], F32)

            def do_bucket(psum1, G, it, dt_, nt_, b, T, first, last):
                """All matmuls of one bucket into psum1."""
                for t in range(T):
                    S = spool.tile([P, ps], BF16)
                    col = b * T + t
                    nc.vector.tensor_scalar(
                        S[:], iota_t, dt_[:, col:col + 1], nt_[:, col:col + 1],
                        mybir.AluOpType.is_equal, mybir.AluOpType.mult)
                    nc.tensor.matmul(
                        psum1[:], G[:, t, :], S[:],
                        start=(first and t == 0), stop=(last and t == T - 1),
                        skip_group_check=True)

            qctr = [0]

            def gather_bucket(table_ap, it, b, T):
                """Gather one bucket's src rows, chunked so each dma_gather
                stays under the SWDGE descriptor-ring capacity. Calls rotate
                across SWDGE queues so desc-gen runs on all Q7 core pairs."""
                G = gpool.tile([P, max(T1, T2), f], BF16, tag="G")
                insts = []
                for t0 in range(0, T, GATHER_CHUNK):
                    tn = min(GATHER_CHUNK, T - t0)
                    q = qctr[0] % N_QUEUES
                    qctr[0] += 1
                    insts.append(nc.gpsimd.dma_gather(
                        G[:, t0:t0 + tn, :], table_ap,
                        it[:, (b * T + t0) * 8:(b * T + t0 + tn) * 8],
                        num_idxs=tn * P, num_idxs_reg=tn * P, elem_size=f,
                        queue_num=q))
                return G, insts

            def flush_pair(agg, layer, p, out_dram):
                for k in range(ps // P):
                    psum2 = psum2_pool.tile([P, f], F32)
                    nc.tensor.matmul(
                        psum2[:], agg[:, k * P:(k + 1) * P], w_t[layer],
                        start=True, stop=True, skip_group_check=True)
                    ob = opool.tile([P, f], F32, tag="ob")
                    nc.vector.tensor_tensor(
                        ob[:], psum2[:], bb_t[layer], mybir.AluOpType.add)
                    odt = BF16 if layer == 0 else F32
                    o2 = opool.tile([P, f], odt, tag="o2" + str(layer))
                    nc.scalar.activation(
                        o2[:], ob[:], mybir.ActivationFunctionType.Relu)
                    r0 = p * ps + k * P
                    nc.sync.dma_start(out_dram[r0:r0 + P, :], o2[:])

            # ---------------- layer 1 ----------------
            xh = [xbf[0:cfg.half_id, :], xbf[cfg.half_id:cfg.n_nodes, :]]
            for p in range(npair):
                psum1 = psum1_pool.tile([P, ps], F32)
                for h in range(2):
                    b = p * 2 + h
                    G, gis = gather_bucket(xh[h], i1t, b, T1)
                    for gi in gis:
                        bass._add_dep_helper(gi.ins, cast_insts[h].ins,
                                             reason="gather after x cast")
                    do_bucket(psum1, G, i1t, d1t, n1t, b, T1,
                              first=(h == 0), last=(h == 1))
                agg = apool.tile([P, ps], BF16, tag="agg")
                nc.vector.tensor_copy(agg[:], psum1[:])
                flush_pair(agg, 0, p, ag_in)

            ag_inst = nc.gpsimd.collective_compute(
                "AllGather", mybir.AluOpType.bypass, replica_groups=groups,
                ins=[ag_in[:, :]], outs=[tab[:, :]])

            # ---------------- layer 2 ----------------
            tabh = [tab[0:cfg.half_rows2, :], tab[cfg.half_rows2:cfg.total_slots, :]]
            for h in range(2):
                for p in range(npair):
                    b = h * npair + p
                    psum1 = psum1_pool.tile([P, ps], F32)
                    G, gis = gather_bucket(tabh[h], i2t, b, T2)
                    for gi in gis:
                        bass._add_dep_helper(gi.ins, ag_inst.ins,
                                             reason="gather after AG")
                    do_bucket(psum1, G, i2t, d2t, n2t, b, T2,
                              first=True, last=True)
                    if h == 0:
                        nc.vector.tensor_copy(
                            accum[:, p * ps:(p + 1) * ps], psum1[:])
                    else:
                        agg = apool.tile([P, ps], BF16, tag="agg")
                        nc.vector.tensor_tensor(
                            agg[:], psum1[:], accum[:, p * ps:(p + 1) * ps],
                            mybir.AluOpType.add)
                        flush_pair(agg, 1, p, z)

    nc.compile()
    return nc


# ---------------- top level ----------------

def run(cfg, x, edge_index, W1, b1, W2, b2, trace=False):
    pl = plan(cfg, edge_index)
    nc = build(cfg, pl["T1"], pl["T2"])
    cbf, cf32 = make_consts(cfg, np.asarray(W1, np.float32),
                            np.asarray(W2, np.float32),
                            np.asarray(b1, np.float32), np.asarray(b2, np.float32))
    x = np.ascontiguousarray(np.asarray(x, np.float32))
    in_maps = []
    for c in range(cfg.n_cores):
        in_maps.append({
            "x": x,
            "idx1": np.ascontiguousarray(pl["idx1"][c]),
            "dest1": np.ascontiguousarray(pl["dest1"][c]),
            "norm1": np.ascontiguousarray(pl["norm1"][c]),
            "idx2": np.ascontiguousarray(pl["idx2"][c]),
            "dest2": np.ascontiguousarray(pl["dest2"][c]),
            "norm2": np.ascontiguousarray(pl["norm2"][c]),
            "cbf": cbf,
            "cf32": cf32,
        })
    res = run_bass_kernel_spmd(nc, in_maps, list(range(cfg.n_cores)), trace=trace)
    out = np.empty((cfg.n_nodes, cfg.feat), np.float32)
    for c in range(cfg.n_cores):
        zc = res.results[c]["z"]
        sel = pl["node_of_slot"][c]
        valid = sel >= 0
        out[sel[valid]] = zc[valid]
    return out, res


def kernel(x, edge_index, W1, b1, W2, b2):
    cfg = Cfg(N_NODES, N_CORES)
    out, _ = run(cfg, x, edge_index, W1, b1, W2, b2, trace=False)
    return out

